# revision 20
# baseline (speedup 1.0000x reference)
"""Trainium2 Bass kernel for nn_EntityPredictor (bidirectional span LSTM entity scorer).

Strategy (8 NeuronCores, data-parallel over spans):
  - Host: sort spans by length desc, pad with dummy spans so every core gets an
    identical length histogram (n=spans/core, schedule n_t = #spans with len>t,
    all even).  Spans are dealt round-robin to the 8 cores, so one compiled SPMD
    program serves all cores.
  - Device (per core), "transposed" layout (gate/hidden dim on partitions, spans
    on the free dim):
      1. Indirect-DMA gather of the valid tokens (t-major staircase) from the
         full hidden_layers table resident in DRAM.
      2. PE-transpose the gathered [tok, 768] tiles into xT [768, tok] (bf16).
      3. XG = W_ih'^T.T @ xT for both directions (bf16 matmuls into PSUM), then
         ACT-Identity copies PSUM -> SBUF bf16 with the per-gate bias fused.
      4. Recurrence, both directions interleaved step-by-step so PE/ACT/DVE
         pipeline across directions.  Per step and direction: PE computes
         identity-add of the XG slice + W_hh'^T.T @ h into a packed 3-bank PSUM
         tile; one merged ACT applies sigmoid to the i/f/o chunks and one the
         tanh to g; DVE updates c (fp32), h (bf16 slab, directly in matmul-rhs
         layout) and the masked time-sum of h (fp32).  Forward walks t
         ascending; backward walks t DESCENDING, which makes the x_rev reversal
         free (every span consumes token t at global step t in both directions).
      5. logits^T [16, n] = E'^T.T @ [hsum_f; hsum_b] (fp32) and DMA out.
  - Host: unpermute spans, drop dummies, return [1280, 16] float32.

Gate padding: each gate block (200) is padded to 256 so gate boundaries align
with 128-partition chunks; gate order is permuted to (i, f, o, g) so chunks 0-5
are sigmoid and chunks 6-7 tanh.  Padded rows produce h=0 and contribute nothing.
"""

import numpy as np

BSZ, SEQ, D, H, L = 64, 512, 768, 200, 5
N_ENT = 16
NCORES = 8
HP = 256          # padded gate block
G4 = 4 * HP       # 1024 padded gate rows
KT = [(0, 128), (128, 72)]  # hidden-dim k-tiles (200 rows)

_CACHE = {}


# ---------------------------------------------------------------- host planning

def _plan(span_len, span_batch, span_token_idx):
    lens = np.asarray(span_len).astype(np.int64)
    NS = lens.shape[0]
    flat = (np.asarray(span_batch).astype(np.int64)[:, None] * SEQ
            + np.asarray(span_token_idx).astype(np.int64))       # [NS, L]
    hist = np.bincount(lens, minlength=L + 1)[1:]
    # pad classes to multiples of 16 -> per-core class counts even -> even n_t
    pad = (-hist) % (2 * NCORES)
    all_len = np.concatenate([lens, np.repeat(np.arange(1, L + 1), pad)])
    order = np.argsort(-all_len, kind="stable")                  # length desc
    N = all_len.shape[0]
    n = N // NCORES
    cores = [order[k::NCORES] for k in range(NCORES)]            # [n] ids, len desc
    n_t = tuple(int((all_len[cores[0]] > t).sum()) for t in range(L))
    for k in range(1, NCORES):
        assert tuple(int((all_len[cores[k]] > t).sum()) for t in range(L)) == n_t
    assert all(w % 2 == 0 for w in n_t) and n % 2 == 0
    offs = np.concatenate([[0], np.cumsum(n_t)]).astype(int)
    ntok = int(offs[-1])
    nchunk = (ntok + 127) // 128
    ntok_pad = nchunk * 128
    return dict(cores=cores, n=n, n_t=n_t, offs=offs, ntok=ntok,
                nchunk=nchunk, ntok_pad=ntok_pad, NS=NS,
                n_pad_spans=int(pad.sum()))


def _gidx(inputs, meta):
    flat = (np.asarray(inputs["span_batch"]).astype(np.int64)[:, None] * SEQ
            + np.asarray(inputs["span_token_idx"]).astype(np.int64))
    all_flat = np.concatenate(
        [flat, np.zeros((meta["n_pad_spans"], L), np.int64)])
    n_t, ntok_pad, nchunk = meta["n_t"], meta["ntok_pad"], meta["nchunk"]
    out = np.zeros((NCORES, 128, nchunk), np.int32)
    for k in range(NCORES):
        ids = meta["cores"][k]
        lst = np.concatenate([all_flat[ids[: n_t[t]], t] for t in range(L)])
        buf = np.zeros(ntok_pad, np.int64)
        buf[: meta["ntok"]] = lst
        out[k] = buf.reshape(nchunk, 128).T.astype(np.int32)
    return out


def _prep_dir(W_ih, W_hh, b_ih, b_hh):
    """Pad gates 200->256, permute gate order (i,f,g,o)->(i,f,o,g), transpose."""
    import ml_dtypes
    W_ih = np.asarray(W_ih, np.float32)
    W_hh = np.asarray(W_hh, np.float32)
    b = np.asarray(b_ih, np.float32) + np.asarray(b_hh, np.float32)
    A = np.zeros((G4, D), np.float32)
    B = np.zeros((G4, H), np.float32)
    bias = np.zeros((G4,), np.float32)
    for newg, oldg in enumerate([2, 0, 1, 3]):                   # g, i, f, o
        dst = slice(newg * HP, newg * HP + H)
        src = slice(oldg * H, (oldg + 1) * H)
        A[dst] = W_ih[src]
        B[dst] = W_hh[src]
        bias[dst] = b[src]
    # Packed for single-DMA loads:
    #   A2 [128, 6*1024]: cols [k*1024:(k+1)*1024] = A^T rows [k*128:(k+1)*128]
    #   B2 [128, 2*1024]: k-tile 0 full, k-tile 1 in rows 0:72
    AT = A.T.astype(ml_dtypes.bfloat16)          # [768, 1024]
    A2 = np.zeros((128, 6 * G4), ml_dtypes.bfloat16)
    for k in range(6):
        A2[:, k * G4:(k + 1) * G4] = AT[k * 128:(k + 1) * 128]
    BT = B.T.astype(ml_dtypes.bfloat16)          # [200, 1024]
    B2 = np.zeros((128, 2 * G4), ml_dtypes.bfloat16)
    B2[:, 0:G4] = BT[0:128]
    B2[:72, G4:2 * G4] = BT[128:200]
    return A2, B2, bias.reshape(8, 128).T.copy()


# ---------------------------------------------------------------- device program

def _build(meta):
    import concourse.bacc as bacc
    import concourse.bass as bass
    import concourse.mybir as mybir
    import concourse.tile as tile
    from concourse.masks import make_identity

    f32 = mybir.dt.float32
    bf16 = mybir.dt.bfloat16
    i32 = mybir.dt.int32
    AF = mybir.ActivationFunctionType
    n, n_t, offs, ntok = meta["n"], meta["n_t"], meta["offs"], meta["ntok"]
    nchunk, ntok_pad = meta["nchunk"], meta["ntok_pad"]

    nc = bacc.Bacc("TRN2", target_bir_lowering=False, debug=False,
                   num_devices=NCORES)
    table = nc.dram_tensor("table", [BSZ * SEQ, D], bf16, kind="ExternalInput").ap()
    gidx_d = nc.dram_tensor("gidx", [128, nchunk], i32, kind="ExternalInput").ap()
    A_d = {dd: nc.dram_tensor(f"A_{dd}", [128, 6 * G4], bf16,
                              kind="ExternalInput").ap() for dd in "fb"}
    B_d = {dd: nc.dram_tensor(f"B_{dd}", [128, 2 * G4], bf16,
                              kind="ExternalInput").ap() for dd in "fb"}
    bias_d = nc.dram_tensor("bias", [128, 16], f32, kind="ExternalInput").ap()
    et_d = nc.dram_tensor("ET", [128, 4 * N_ENT], f32, kind="ExternalInput").ap()
    out_d = nc.dram_tensor("out", [N_ENT, n], f32, kind="ExternalOutput").ap()
    idf_d = nc.dram_tensor("identf", [128, 128], f32, kind="ExternalInput").ap()
    idb_d = nc.dram_tensor("identb", [128, 128], bf16, kind="ExternalInput").ap()

    with tile.TileContext(nc) as tc:
        with tc.tile_pool(name="const", bufs=1) as const, \
             tc.tile_pool(name="gx", bufs=3) as gxp, \
             tc.tile_pool(name="state", bufs=1) as statep, \
             tc.tile_pool(name="work", bufs=2) as workp:

            idx_sb = const.tile([128, nchunk], i32)
            nc.sync.dma_start(out=idx_sb[:], in_=gidx_d[:, :])
            ident = const.tile([128, 128], f32)
            nc.sync.dma_start(out=ident[:], in_=idf_d[:, :])
            identb = const.tile([128, 128], bf16)
            nc.sync.dma_start(out=identb[:], in_=idb_d[:, :])
            bias_sb = const.tile([128, 16], f32)
            nc.sync.dma_start(out=bias_sb[:], in_=bias_d[:, :])
            # preload the sigmoid/tanh ACT table set out of the critical path
            actwarm = const.tile([1, 4], f32)
            nc.scalar.activation(actwarm[:, 0:2], bias_sb[0:1, 0:2], AF.Sigmoid)
            nc.scalar.activation(actwarm[:, 2:4], bias_sb[0:1, 0:2], AF.Tanh)
            et2 = const.tile([128, 4 * N_ENT], f32)
            nc.sync.dma_start(out=et2[:], in_=et_d[:, :])
            et_tiles = [et2[:pl, j * N_ENT:(j + 1) * N_ENT]
                        for j, pl in enumerate([128, 72, 128, 72])]
            A_sb, B_sb = {}, {}
            for dd in "fb":
                a2 = const.tile([128, 6 * G4], bf16, tag=f"A{dd}")
                nc.sync.dma_start(out=a2[:], in_=A_d[dd][:, :])
                A_sb[dd] = [a2[:, k * G4:(k + 1) * G4] for k in range(6)]
            for dd in "fb":
                b2 = const.tile([128, 2 * G4], bf16, tag=f"B{dd}")
                nc.sync.dma_start(out=b2[:], in_=B_d[dd][:, :])
                B_sb[dd] = [b2[:128, 0:G4], b2[:72, G4:2 * G4]]

            # ---- gather + transpose -> xT[k] = x^T [768, ntok_pad] bf16
            xT = [const.tile([128, ntok_pad], bf16, tag=f"xT{k}", name=f"xT{k}")
                  for k in range(6)]
            warmsrc = const.tile([128, 128], bf16)
            nc.vector.memset(warmsrc[:], 0.0)
            with tc.tile_pool(name="wp", bufs=1, space="PSUM") as wpp, \
                 tc.tile_pool(name="tp", bufs=2, space="PSUM") as tpp:
                warm = wpp.tile([128, 128], f32, tag="warm")
                for wi in range(48):
                    nc.tensor.matmul(warm[:], warmsrc[:], warmsrc[:],
                                     start=True, stop=True)
                for c in range(nchunk):
                    gx = gxp.tile([128, D], bf16)
                    nc.gpsimd.indirect_dma_start(
                        out=gx[:], out_offset=None, in_=table[:, :],
                        in_offset=bass.IndirectOffsetOnAxis(
                            ap=idx_sb[:, c:c + 1], axis=0))
                    for k in range(6):
                        pt = tpp.tile([128, 128], bf16)
                        nc.tensor.transpose(pt[:], gx[:, k * 128:(k + 1) * 128],
                                            identb[:])
                        nc.vector.tensor_copy(
                            out=xT[k][:, c * 128:(c + 1) * 128], in_=pt[:])
                    # HAM fillers: keep PE "busy" while waiting on gathers
                    for wi in range(12):
                        nc.tensor.matmul(warm[:], warmsrc[:], warmsrc[:],
                                         start=True, stop=True)
                for wi in range(30):
                    nc.tensor.matmul(warm[:], warmsrc[:], warmsrc[:],
                                     start=True, stop=True)

            # ---- XG for both directions -> SBUF bf16 (bias fused in the copy)
            xg_sb = {dd: [const.tile([128, ntok], bf16, tag=f"xgsb{dd}{m}",
                                     name=f"xgsb{dd}{m}")
                          for m in range(8)] for dd in "fb"}
            with tc.tile_pool(name="xgp", bufs=1, space="PSUM") as xgp:
                for di, dd in enumerate("fb"):
                    for m in range(8):
                        xg = xgp.tile([128, ntok], f32, tag=f"xg{m}",
                                      name=f"xg{dd}{m}")
                        for k in range(6):
                            nc.tensor.matmul(
                                xg[:, :],
                                A_sb[dd][k][:, m * 128:(m + 1) * 128],
                                xT[k][:, :ntok],
                                start=(k == 0), stop=(k == 5))
                        nc.scalar.activation(
                            xg_sb[dd][m][:, :], xg[:, :], AF.Identity,
                            bias=bias_sb[:, di * 8 + m: di * 8 + m + 1])

            # ---- merged-direction recurrence: per global step si, forward
            # step t=si and backward step t=L-1-si share one PSUM tile; chunk m
            # holds [f-part w_f | b-part w_b] packed 2 chunks per bank.
            hsl = {}   # h slab  [128, 2n] bf16   (block j at cols [j*n, j*n+w))
            csl = {}   # c slab  [128, 2n] f32
            hss = {}   # hsum    [128, 2n] f32
            for dd in "fb":
                hsl[dd] = statep.tile([128, 2 * n], bf16, name=f"hsl{dd}")
                csl[dd] = statep.tile([128, 2 * n], f32, name=f"csl{dd}")
                hss[dd] = statep.tile([128, 2 * n], f32, name=f"hss{dd}")
                nc.vector.memset(hsl[dd][:], 0.0)
                nc.vector.memset(csl[dd][:], 0.0)
                nc.vector.memset(hss[dd][:], 0.0)

            def blk2(tile_ap, w):
                # [128, 2, w] strided view of a [128, 2n] slab (blocks at 0, n)
                return tile_ap.rearrange("p (b q) -> p b q", b=2)[:, :, :w]

            with tc.tile_pool(name="pre", bufs=1, space="PSUM") as prep:
                geom = []
                for si in range(L):
                    tf, tb = si, L - 1 - si
                    wf, wb = n_t[tf], n_t[tb]
                    geom.append(dict(
                        ws=wf + wb,
                        wd={"f": wf, "b": wb},
                        od={"f": int(offs[tf]), "b": int(offs[tb])},
                        sh={"f": 0, "b": wf}))
                pres = {}

                def chunk(si, m, dd):
                    g = geom[si]
                    off = (m % 2) * g["ws"] + g["sh"][dd]
                    return pres[si][m // 2][:, off:off + g["wd"][dd]]

                def ident_mms(si):
                    # identity-add of the XG slices opens each bank's group;
                    # emitted one step ahead as PE filler during step si-1.
                    g = geom[si]
                    pres[si] = [prep.tile([128, 512], f32,
                                          tag=f"pre{si % 2}b{bank}",
                                          name=f"pre{si}b{bank}")
                                for bank in range(4)]
                    for dd in "fb":
                        for m in range(8):
                            nc.tensor.matmul(
                                chunk(si, m, dd), identb[:],
                                xg_sb[dd][m][:, g["od"][dd]:g["od"][dd] + g["wd"][dd]],
                                start=(dd == "f" and m % 2 == 0), stop=False)

                ident_mms(0)
                for si in range(L):
                    g = geom[si]
                    ws, wd, sh = g["ws"], g["wd"], g["sh"]
                    pre = pres[si]
                    # B-matmuls bank-major (bank0=g, 1=i, 2=f, 3=o) so ACT can
                    # fire per-bank as soon as that bank's group stops.
                    for bank in range(4):
                        for dd in "fb":
                            for j, (p0, pl) in enumerate(KT):
                                for m in (2 * bank, 2 * bank + 1):
                                    nc.tensor.matmul(
                                        chunk(si, m, dd),
                                        B_sb[dd][j][:, m * 128:(m + 1) * 128],
                                        hsl[dd][:pl, j * n:j * n + wd[dd]],
                                        start=False,
                                        stop=(dd == "b" and j == 1 and m % 2 == 1))
                    if si + 1 < L:
                        ident_mms(si + 1)
                        # HAM fillers: junk N=2 matmuls into unused tail bytes
                        # of the next step's bank-3 tile (never read)
                        for wi in range(20):
                            nc.tensor.matmul(
                                pres[si + 1][3][:, 500:502], warmsrc[:],
                                warmsrc[:, 0:2], start=False, stop=False,
                                skip_group_check=True)
                    # per-bank activations (cover both directions at once)
                    gtan = workp.tile([128, 2 * ws], bf16, tag="gtan",
                                      name=f"gtan{si}")
                    gi = workp.tile([128, 2 * ws], bf16, tag="gi", name=f"gi{si}")
                    gf = workp.tile([128, 2 * ws], bf16, tag="gf", name=f"gf{si}")
                    go = workp.tile([128, 2 * ws], bf16, tag="go", name=f"go{si}")
                    nc.scalar.activation(gtan[:], pre[0][:, 0:2 * ws], AF.Tanh)
                    nc.scalar.activation(gi[:], pre[1][:, 0:2 * ws], AF.Sigmoid)
                    nc.scalar.activation(gf[:], pre[2][:, 0:2 * ws], AF.Sigmoid)
                    nc.scalar.activation(go[:], pre[3][:, 0:2 * ws], AF.Sigmoid)

                    views = {}
                    for dd in "fb":
                        w = wd[dd]
                        lo, hi = sh[dd], sh[dd] + w
                        views[dd] = dict(
                            g=gtan[:].rearrange("p (c q) -> p c q", c=2)[:, :, lo:hi],
                            i=gi[:].rearrange("p (c q) -> p c q", c=2)[:, :, lo:hi],
                            f=gf[:].rearrange("p (c q) -> p c q", c=2)[:, :, lo:hi],
                            o=go[:].rearrange("p (c q) -> p c q", c=2)[:, :, lo:hi],
                            c=blk2(csl[dd], w), h=blk2(hsl[dd], w),
                            s=blk2(hss[dd], w))
                        t1 = workp.tile([128, 2 * w], bf16, tag=f"t1{dd}",
                                        name=f"t1{dd}{si}")
                        tc_ = workp.tile([128, 2 * w], bf16, tag=f"tc{dd}",
                                         name=f"tc{dd}{si}")
                        views[dd]["t1"] = t1[:].rearrange("p (b q) -> p b q", b=2)
                        views[dd]["tc"] = tc_[:].rearrange("p (b q) -> p b q", b=2)
                    # c-chains first (f then b), tanh_c fired as soon as each c
                    # is ready, h muls afterwards so DVE stays busy during tanh
                    for dd in "fb":
                        v = views[dd]
                        nc.vector.tensor_mul(v["t1"], v["i"], v["g"])
                        nc.vector.tensor_mul(v["c"], v["c"], v["f"])
                        nc.vector.tensor_add(v["c"], v["c"], v["t1"])
                        nc.scalar.activation(v["tc"], v["c"], AF.Tanh)
                    for dd in "fb":
                        v = views[dd]
                        nc.vector.tensor_mul(v["h"], v["o"], v["tc"])
                    for dd in "fb":
                        v = views[dd]
                        nc.vector.tensor_add(v["s"], v["s"], v["h"])

            # ---- logits^T = E'^T.T @ [hsum_f; hsum_b]
            with tc.tile_pool(name="lg", bufs=1, space="PSUM") as lgp:
                lg = lgp.tile([N_ENT, n], f32)
                rhs = [hss["f"][:, 0:n], hss["f"][:72, n:2 * n],
                       hss["b"][:, 0:n], hss["b"][:72, n:2 * n]]
                for j4 in range(4):
                    nc.tensor.matmul(lg[:, :], et_tiles[j4], rhs[j4],
                                     start=(j4 == 0), stop=(j4 == 3))
                out_sb = const.tile([N_ENT, n], f32)
                nc.vector.tensor_copy(out=out_sb[:], in_=lg[:, :])
                nc.sync.dma_start(out=out_d[:, :], in_=out_sb[:])

    nc.compile()
    return nc


# ---------------------------------------------------------------- entry points

def run(inputs, trace=False, trace_cores=None):
    from concourse.bass_utils import run_bass_kernel_spmd

    meta = _plan(inputs["span_len"], inputs["span_batch"],
                 inputs["span_token_idx"])
    key = (meta["n"], meta["n_t"], meta["ntok"])
    if key not in _CACHE:
        _CACHE[key] = _build(meta)
    nc = _CACHE[key]

    A_f, B_f, bias_f = _prep_dir(inputs["W_ih_f"], inputs["W_hh_f"],
                                 inputs["b_ih_f"], inputs["b_hh_f"])
    A_b, B_b, bias_b = _prep_dir(inputs["W_ih_b"], inputs["W_hh_b"],
                                 inputs["b_ih_b"], inputs["b_hh_b"])
    bias = np.concatenate([bias_f, bias_b], axis=1)
    ETt = np.asarray(inputs["entity_embs"], np.float32).T     # [400, 16]
    ET = np.zeros((128, 4 * N_ENT), np.float32)
    for j, (p0, pl) in enumerate([(0, 128), (128, 72), (200, 128), (328, 72)]):
        ET[:pl, j * N_ENT:(j + 1) * N_ENT] = ETt[p0:p0 + pl]
    import ml_dtypes
    table = np.ascontiguousarray(
        np.asarray(inputs["hidden_layers"], np.float32)
        .reshape(BSZ * SEQ, D).astype(ml_dtypes.bfloat16))
    gidx_all = _gidx(inputs, meta)
    identf = np.eye(128, dtype=np.float32)
    identb = np.eye(128).astype(ml_dtypes.bfloat16)

    in_maps = [dict(table=table, gidx=gidx_all[k], A_f=A_f, A_b=A_b,
                    B_f=B_f, B_b=B_b, bias=bias, ET=ET,
                    identf=identf, identb=identb)
               for k in range(NCORES)]
    res = run_bass_kernel_spmd(nc, in_maps, list(range(NCORES)),
                               trace=trace, trace_cores=trace_cores)
    n, NS = meta["n"], meta["NS"]
    logits = np.zeros((NS, N_ENT), np.float32)
    for k in range(NCORES):
        outk = res.results[k]["out"]                              # [16, n]
        ids = meta["cores"][k]
        sel = ids < NS
        logits[ids[sel]] = outk[:, sel].T
    return logits, res


def kernel(**inputs):
    logits, _ = run(inputs, trace=False)
    return logits
